# revision 1
# baseline (speedup 1.0000x reference)
"""KG scoring kernel: scores[b,e] = W2 . relu([h,r,t] MLP) over all entities,
sharded across 8 TRN2 NeuronCores along the entity axis (sharded-ANN pattern).

Math restructuring (exact, fp32):
  scores[b,e] = sum_h W2[h] * relu(ph[b,h] + pt[e,h]) + b2
              = sum_h sgn[h] * relu(|W2[h]|*pt[e,h] + |W2[h]|*ph[b,h]) + b2
so |W2| is folded into the W1t matmul weights (host-side) and the per-(b,h)
bias; the reduction over h becomes a matmul with the sign vector (M=1).
b2 is a constant shift -> added on host after top-k (ranking invariant).

Each core: local scores for its 6250 entities -> local top-16 per
(batch, entity-group of 392) via DVE max8/match_replace -> host re-selects
exact global top-k from the gathered candidates.
"""

import numpy as np

import concourse.bass as bass
import concourse.bacc as bacc
import concourse.tile as tile
from concourse import mybir
from concourse import bass_utils

B = 8           # batch
E = 50000       # entities
D = 128         # embedding dim
H = 256         # hidden dim
NCORES = 8
E_LOC = E // NCORES          # 6250 entities per core
G = 16                       # score groups per core (one per SBUF partition set)
W = 392                      # entities per group
E_PAD = G * W                # 6272 (padded shard size)
NEG = -1.0e30

TRACE = False                # test.py sets this to profile
LAST_RESULTS = None          # BassKernelResults of the last run

_cache = {}


def _build_nc():
    nc = bacc.Bacc("TRN2", target_bir_lowering=False, debug=False)
    f32 = mybir.dt.float32
    u32 = mybir.dt.uint32
    AF = mybir.ActivationFunctionType
    OP = mybir.AluOpType

    entT = nc.dram_tensor("entT", [D, E_PAD], f32, kind="ExternalInput")
    w1ts = nc.dram_tensor("w1ts", [D, H], f32, kind="ExternalInput")
    qb = nc.dram_tensor("qb", [128, 16], f32, kind="ExternalInput")
    sgn = nc.dram_tensor("sgn", [128, 2, 32], f32, kind="ExternalInput")
    cand_val = nc.dram_tensor("cand_val", [128, 16], f32, kind="ExternalOutput")
    cand_idx = nc.dram_tensor("cand_idx", [128, 16], f32, kind="ExternalOutput")

    # which of the 16 (b,c) relu ops run on ACT (rest on DVE)
    act_relu = {0, 3, 5, 8, 11, 13, 15}

    with tile.TileContext(nc) as tc:
        with (
            tc.tile_pool(name="consts", bufs=1) as consts,
            tc.tile_pool(name="ent", bufs=G) as entp,
            tc.tile_pool(name="ptsb", bufs=1) as ptsbp,
            tc.tile_pool(name="u", bufs=16) as up,
            tc.tile_pool(name="sc", bufs=1) as scp,
            tc.tile_pool(name="pt_ps", bufs=2, space=bass.MemorySpace.PSUM) as pt_ps,
            tc.tile_pool(name="sc_ps", bufs=6, space=bass.MemorySpace.PSUM) as sc_ps,
            tc.tile_pool(name="dram", bufs=1, space="DRAM") as dramp,
        ):
            w1ts_sb = consts.tile([D, H], f32, tag="w1ts")
            nc.sync.dma_start(w1ts_sb[:], w1ts[:])
            qb_sb = consts.tile([128, 16], f32, tag="qb")
            nc.sync.dma_start(qb_sb[:], qb[:])
            sgn_sb = consts.tile([128, 2, 32], f32, tag="sgn")
            nc.sync.dma_start(sgn_sb[:], sgn[:])

            # ---- load entity shard (transposed [D, E_PAD]) in G chunks ----
            ent_tiles = []
            for g in range(G):
                t = entp.tile([D, W], f32, tag="ent")
                nc.sync.dma_start(t[:], entT[:, g * W:(g + 1) * W])
                ent_tiles.append(t)

            # ---- ptT[c][h,e] = (W1t*|W2|).T @ entT, resident in SBUF ----
            pt_sb = [ptsbp.tile([128, E_PAD], f32, tag=f"pt{c}", name=f"pt{c}")
                     for c in range(2)]
            for g in range(G):
                for c in range(2):
                    ps_full = pt_ps.tile([128, 512], f32, tag="ptps",
                                         name="ps_full")
                    ps = ps_full[:, :W]
                    nc.tensor.matmul(
                        ps, w1ts_sb[:, c * 128:(c + 1) * 128], ent_tiles[g][:],
                        start=True, stop=True,
                    )
                    dst = pt_sb[c][:, g * W:(g + 1) * W]
                    if (g * 2 + c) % 2 == 0:
                        nc.scalar.activation(dst, ps, AF.Copy)
                    else:
                        nc.vector.tensor_copy(dst, ps)

            # ---- scores2[p = 16*b + g, :] ----
            # Entity range processed in blocks of GB groups. Within a block,
            # relu for all (b, c) is computed, then the W2-dot packs 4 batches
            # per PSUM tile via tile_position col-groups (partitions 0/32/64/96)
            # so one PSUM->SBUF copy moves 4 score rows; a DMA then remaps
            # partitions into the scores2[(b,g)] top-k layout.
            scores2 = scp.tile([128, W], f32, tag="scores2")
            # stage_all[32j, 16h+g, :] = scores of (b=4h+j, g); partitions
            # 32j come straight from the tile_position col-group outputs.
            stage_all = scp.tile([128, 2 * G, W], f32, tag="stage_all")
            GB = 2                    # groups per block
            BW = GB * W               # block width
            for blk in range(G // GB):
                us = {}
                for b in range(B):
                    for c in range(2):
                        ut = up.tile([128, BW], f32, tag="u", name="ut")
                        col = c * 8 + b
                        bias_ap = qb_sb[:, col:col + 1]
                        src = pt_sb[c][:, blk * BW:(blk + 1) * BW]
                        if (b * 2 + c) in act_relu:
                            nc.scalar.activation(ut[:], src, AF.Relu,
                                                 bias=bias_ap, scale=1.0)
                        else:
                            nc.vector.tensor_scalar(ut[:], src, bias_ap, 0.0,
                                                    OP.add, OP.max)
                        us[(b, c)] = ut
                for gi in range(GB):
                    g = blk * GB + gi
                    for half in range(2):
                        ps_f = sc_ps.tile([128, 512], f32, tag="scps",
                                          name="ps_f")
                        ps = ps_f[:, :W]
                        for j in range(4):
                            b = 4 * half + j
                            # M=32 with zero cols 1..31: row 32j is the real
                            # score, rows 32j+1..32j+31 are written zeros so
                            # the whole bank is initialized for the copy.
                            nc.tensor.matmul(
                                ps[32 * j:32 * j + 32], sgn_sb[:, 0, :],
                                us[(b, 0)][:, gi * W:(gi + 1) * W],
                                start=True, stop=False, tile_position=(0, 32 * j))
                            nc.tensor.matmul(
                                ps[32 * j:32 * j + 32], sgn_sb[:, 1, :],
                                us[(b, 1)][:, gi * W:(gi + 1) * W],
                                start=False, stop=True, tile_position=(0, 32 * j))
                        dst = stage_all[:, G * half + g, :]
                        if (g * 2 + half) % 2 == 0:
                            nc.scalar.activation(dst, ps, AF.Copy)
                        else:
                            nc.vector.tensor_copy(dst, ps)

            # ---- mask the 22 padded entities (group 15, cols 370..391) ----
            nc.vector.memset(stage_all[:, 15:2 * G:G, E_LOC - 15 * W:W], NEG)

            # partition remap: scores2[32j + 16h + g] = stage_all[32j, 16h+g],
            # bounced through DRAM (flat) so it is two big single-writer DMAs.
            bounce = dramp.tile([4, 2 * G, W], f32, tag="bounce")
            nc.sync.dma_start(bounce[:], stage_all[0:97:32])
            nc.sync.dma_start(scores2[:], bounce[:].rearrange("j r w -> (j r) w"))

            # ---- per-partition top-16 via two max8 rounds ----
            tk = scp
            v1 = tk.tile([128, 8], f32, tag="v1")
            i1 = tk.tile([128, 8], u32, tag="i1")
            v2 = tk.tile([128, 8], f32, tag="v2")
            i2 = tk.tile([128, 8], u32, tag="i2")
            s3 = tk.tile([128, W], f32, tag="s3")
            nc.vector.max(v1[:], scores2[:])
            nc.vector.max_index(i1[:], v1[:], scores2[:])
            nc.vector.match_replace(s3[:], v1[:], scores2[:], NEG)
            nc.vector.max(v2[:], s3[:])
            nc.vector.max_index(i2[:], v2[:], s3[:])

            nc.sync.dma_start(cand_val[:, 0:8], v1[:])
            nc.sync.dma_start(cand_val[:, 8:16], v2[:])
            fi1 = tk.tile([128, 8], f32, tag="fi1")
            fi2 = tk.tile([128, 8], f32, tag="fi2")
            nc.vector.tensor_copy(fi1[:], i1[:])
            nc.vector.tensor_copy(fi2[:], i2[:])
            nc.sync.dma_start(cand_idx[:, 0:8], fi1[:])
            nc.sync.dma_start(cand_idx[:, 8:16], fi2[:])

    nc.compile()
    return nc


def host_prep(head, relation, ent_emb, rel_emb, W1, b1, W2):
    """Fold |W2| into the tail weights/bias; shard+transpose the entity table."""
    W1h, W1r, W1t = W1[:D], W1[D:2 * D], W1[2 * D:]
    ph = ent_emb[head] @ W1h + rel_emb[relation] @ W1r + b1      # [B, H]
    absW2 = np.abs(W2)
    sgnW2 = np.sign(W2).astype(np.float32)
    w1ts_np = np.ascontiguousarray(W1t * absW2[None, :])          # [D, H]
    qb_full = ph * absW2[None, :]                                 # [B, H]
    qb_np = np.ascontiguousarray(
        qb_full.T.reshape(2, 128, B).transpose(1, 0, 2).reshape(128, 16))
    sgn_np = np.zeros((128, 2, 32), dtype=np.float32)
    sgn_np[:, :, 0] = sgnW2.reshape(2, 128).T
    shards = []
    for c in range(NCORES):
        shT = np.zeros((D, E_PAD), dtype=np.float32)
        shT[:, :E_LOC] = ent_emb[c * E_LOC:(c + 1) * E_LOC].T
        shards.append(shT)
    return w1ts_np, qb_np, sgn_np, shards


def kernel(head, relation, k, ent_emb, rel_emb, W1, b1, W2, b2):
    head = np.asarray(head)
    relation = np.asarray(relation)
    k = int(k)
    ent_emb = np.asarray(ent_emb, dtype=np.float32)
    rel_emb = np.asarray(rel_emb, dtype=np.float32)
    W1 = np.asarray(W1, dtype=np.float32)
    b1 = np.asarray(b1, dtype=np.float32)
    W2 = np.asarray(W2, dtype=np.float32)
    b2 = np.asarray(b2, dtype=np.float32)
    assert k <= 16, f"kernel supports k<=16, got {k}"

    w1ts_np, qb_np, sgn_np, shards = host_prep(
        head, relation, ent_emb, rel_emb, W1, b1, W2)

    if "nc" not in _cache:
        _cache["nc"] = _build_nc()
    nc = _cache["nc"]

    in_maps = [
        {"entT": shards[c], "w1ts": w1ts_np, "qb": qb_np, "sgn": sgn_np}
        for c in range(NCORES)
    ]
    res = bass_utils.run_bass_kernel_spmd(
        nc, in_maps, core_ids=list(range(NCORES)), trace=TRACE)
    global LAST_RESULTS
    LAST_RESULTS = res

    # ---- host merge: exact top-k from 8 cores x 128 partitions x 16 cands ----
    vals = np.stack([r["cand_val"] for r in res.results])         # [C, 128, 16]
    idxs = np.stack([r["cand_idx"] for r in res.results]).astype(np.int64)

    top_indices = np.empty((B, k), np.int32)
    top_scores = np.empty((B, k), np.float32)
    g_of = np.arange(G)[None, :, None]
    c_of = np.arange(NCORES)[:, None, None]
    for b in range(B):
        pb = 32 * (b % 4) + 16 * (b // 4)
        v = vals[:, pb:pb + G, :]                                 # [C, G, 16]
        li = g_of * W + idxs[:, pb:pb + G, :]                     # local entity id
        ge = (c_of * E_LOC + li).reshape(-1)
        vf = v.reshape(-1)
        valid = (li < E_LOC).reshape(-1)
        vf = vf[valid]
        ge = ge[valid]
        order = np.lexsort((ge, -vf))[:k]
        top_indices[b] = ge[order]
        top_scores[b] = vf[order] + b2[0]

    return top_indices, top_scores

